# revision 17
# baseline (speedup 1.0000x reference)
"""Trainium2 Bass kernel for KroneckerLinear: y = x @ kron(U, V).

Math: with x[t] reshaped to X_t [i1=128, i2=128] (i2 contiguous) and
y[t] reshaped to Y_t [j1=128, j2=128] (j2 contiguous):

    Y_t = U^T @ X_t @ V

Dataflow (fp16 end to end, fp32 PSUM accumulation; the 2e-2 rel-err gate
leaves ~30x margin over fp16 rounding):

  1. x is cast to fp16 on the host.  Each 128-token block is loaded with
     the DMA xbar transpose straight from DRAM: x[t-block, :] (contiguous
     rows in HBM) lands in SBUF as xT[i2, i1, t] - no strided 512B
     descriptors anywhere.
  2. Stage 1 (V), per token:  lhsT = xT[:, :, t] = X_t^T [i2, i1],
     rhs = V [i2, j2]  ->  Z_t [i1, j2] in PSUM; copied/cast (DVE and ACT
     alternate) into zb [i1, t, j2] fp16, contiguous on both sides.
  3. Stage 2 (U), per 4 tokens:  lhsT = U [i1, j1] (shared stationary),
     rhs = zb[:, t4, :] [i1, 512] contiguous moving
     ->  PSUM [j1, (t4, j2)]; copied into ybw [j1, t, j2] fp16.
  4. ybw is stored as contiguous rows into y2[b, j1, t, j2]; the host
     transposes to [t, j1, j2] and upcasts to fp32.

Stage-2 matmuls are emitted with a 3-chunk lag behind stage 1 so the PE
never stalls on the PSUM->SBUF copy of the chunk it consumes.

Sharding: data-parallel over tokens, 256 tokens per core x 8 cores.
"""

import sys

if "/opt/trn_rl_repo" not in sys.path:
    sys.path.insert(0, "/opt/trn_rl_repo")

import numpy as np

import concourse.bacc as bacc
import concourse.mybir as mybir
from concourse import tile
from concourse.bass_utils import run_bass_kernel_spmd

F16 = mybir.dt.float16
F32 = mybir.dt.float32

N_CORES = 8
TOKENS = 2048
D = 16384  # 128 * 128
T_CORE = TOKENS // N_CORES  # 256
BLK = 128  # tokens per block (PE partition width)


def build_nc(n_tokens=T_CORE, lsub=32, quad=8, lag=2, sstore=32):
    """Build + compile the per-core program.

    lsub: tokens per xbar transpose-load call.
    quad: stage-1 matmuls per PSUM tile / tokens per stage-2 matmul.
    lag: stage-2 chunk lag behind stage 1 (PE pipelining).
    sstore: tokens per sub-store.
    """
    assert n_tokens % BLK == 0 and BLK % lsub == 0 and BLK % quad == 0
    assert quad % 4 == 0  # stage-2 works in 4-token (N=512) matmuls
    n_blk = n_tokens // BLK
    nq = BLK // quad
    nc = bacc.Bacc("TRN2", target_bir_lowering=False, debug=False)
    x = nc.dram_tensor("x", [n_tokens, D], F16, kind="ExternalInput")
    u = nc.dram_tensor("u", [128, 128], F16, kind="ExternalInput")
    v = nc.dram_tensor("v", [128, 128], F16, kind="ExternalInput")
    # y2[b, j1, t, j2]; host transposes to [t, j1, j2]
    y2 = nc.dram_tensor("y2", [n_blk, 128, BLK, 128], F16, kind="ExternalOutput")

    with tile.TileContext(nc) as tc:
        with (
            tc.tile_pool(name="const", bufs=1) as cpool,
            tc.tile_pool(name="xT", bufs=2) as xpool,
            tc.tile_pool(name="zb", bufs=2) as zpool,
            tc.tile_pool(name="yw", bufs=2) as ypool,
            tc.tile_pool(name="pa", bufs=2, space="PSUM") as papool,
            tc.tile_pool(name="pb", bufs=2, space="PSUM") as pbpool,
        ):
            u_sb = cpool.tile([128, 128], F16)
            v_sb = cpool.tile([128, 128], F16)
            nc.scalar.dma_start(u_sb[:], u[:])
            nc.scalar.dma_start(v_sb[:], v[:])

            for b in range(n_blk):
                t0 = b * BLK
                # xbar transpose-load: xT[i2, i1, t] = x[t0+t, i1*128+i2]
                # block 0: fine-grained first chunks so the PE starts early;
                # later blocks: coarse chunks (fewer serialized ucode calls
                # on the sync ring)
                xT = xpool.tile([128, 128, BLK], F16)
                if b == 0:
                    subs = [16, 16, 32, 64]
                else:
                    subs = [lsub] * (BLK // lsub)
                soff = 0
                for sl in subs:
                    nc.sync.dma_start(
                        xT[:, :, soff : soff + sl],
                        x[t0 + soff : t0 + soff + sl, :],
                        transpose=True,
                    )
                    soff += sl
                zb = zpool.tile([128, BLK, 128], F16)  # [i1, t, j2]
                yw = ypool.tile([128, BLK, 128], F16)  # [j1, t, j2]

                def stage2(c):
                    # one stage-2 "chunk" = quad tokens, as quad//4 matmuls of
                    # N=512 (PSUM bank limit) sharing one PSUM tile + one copy
                    nmm = quad // 4
                    pb = pbpool.tile([128, nmm, 512], F32)
                    for k in range(nmm):
                        tk = c * quad + k * 4
                        nc.tensor.matmul(
                            pb[:, k, :],
                            lhsT=u_sb[:],
                            rhs=zb[:, tk : tk + 4, :].rearrange(
                                "i1 t j2 -> i1 (t j2)"
                            ),
                            start=True,
                            stop=True,
                        )
                    # both PSUM-capable engines drain half the tile each
                    half = quad // 2
                    dst0 = yw[:, c * quad : c * quad + half, :].rearrange(
                        "j1 t j2 -> j1 (t j2)"
                    )
                    dst1 = yw[:, c * quad + half : (c + 1) * quad, :].rearrange(
                        "j1 t j2 -> j1 (t j2)"
                    )
                    flat = pb[:].rearrange("p a b -> p (a b)")
                    nc.scalar.copy(dst0, flat[:, : half * 128])
                    nc.vector.tensor_copy(dst1, flat[:, half * 128 :])
                    # sub-store once a full sstore-token span of yw is done
                    tend = (c + 1) * quad
                    if tend % sstore == 0:
                        s0 = tend - sstore
                        nc.gpsimd.dma_start(
                            y2[b, :, s0:tend, :], yw[:, s0:tend, :]
                        )

                for q in range(nq):
                    pa = papool.tile([128, quad, 128], F32)
                    for j in range(quad):
                        nc.tensor.matmul(
                            pa[:, j, :],
                            lhsT=xT[:, :, q * quad + j],
                            rhs=v_sb[:],
                            start=True,
                            stop=True,
                        )
                    half = quad // 2
                    nc.vector.tensor_copy(
                        zb[:, q * quad : q * quad + half, :], pa[:, :half, :]
                    )
                    nc.scalar.copy(
                        zb[:, q * quad + half : (q + 1) * quad, :],
                        pa[:, half:, :],
                    )
                    if q >= lag:
                        stage2(q - lag)
                for c in range(nq - lag, nq):
                    stage2(c)
    nc.compile()
    return nc


_NC_CACHE = {}


def _get_nc(**kw):
    key = tuple(sorted(kw.items()))
    if key not in _NC_CACHE:
        _NC_CACHE[key] = build_nc(**kw)
    return _NC_CACHE[key]


def run(x, U, V, lsub=32, quad=8, lag=2, sstore=32, trace=False, **spmd_kwargs):
    """Shard over 8 cores, run, gather. Returns (y_full, BassKernelResults)."""
    x = np.asarray(x).astype(np.float16)
    U = np.asarray(U).astype(np.float16)
    V = np.asarray(V).astype(np.float16)
    t_core = x.shape[0] // N_CORES
    nc = _get_nc(n_tokens=t_core, lsub=lsub, quad=quad, lag=lag, sstore=sstore)
    in_maps = [
        {"x": x[i * t_core : (i + 1) * t_core], "u": U, "v": V}
        for i in range(N_CORES)
    ]
    res = run_bass_kernel_spmd(
        nc, in_maps, list(range(N_CORES)), trace=trace, **spmd_kwargs
    )
    # y2[b, j1, t, j2] -> y[t, (j1 j2)]
    out = np.concatenate([res.results[i]["y2"] for i in range(N_CORES)], axis=0)
    out = (
        out.transpose(0, 2, 1, 3)
        .reshape(TOKENS, D)
        .astype(np.float32)
    )
    return out, res


def kernel(x, U, V):
    out, _ = run(x, U, V)
    return out


# revision 18
# speedup vs baseline: 1.1002x; 1.1002x over previous
"""Trainium2 Bass kernel for KroneckerLinear: y = x @ kron(U, V).

Math: with x[t] reshaped to X_t [i1=128, i2=128] (i2 contiguous) and
y[t] reshaped to Y_t [j1=128, j2=128] (j2 contiguous):

    Y_t = U^T @ X_t @ V

Dataflow (fp16 end to end, fp32 PSUM accumulation; the 2e-2 rel-err gate
leaves ~30x margin over fp16 rounding):

  1. x is cast to fp16 on the host.  Each 128-token block is loaded with
     the DMA xbar transpose straight from DRAM: x[t-block, :] (contiguous
     rows in HBM) lands in SBUF as xT[i2, i1, t] - no strided 512B
     descriptors anywhere.
  2. Stage 1 (V), per token:  lhsT = xT[:, :, t] = X_t^T [i2, i1],
     rhs = V [i2, j2]  ->  Z_t [i1, j2] in PSUM; copied/cast (DVE and ACT
     alternate) into zb [i1, t, j2] fp16, contiguous on both sides.
  3. Stage 2 (U), per 4 tokens:  lhsT = U [i1, j1] (shared stationary),
     rhs = zb[:, t4, :] [i1, 512] contiguous moving
     ->  PSUM [j1, (t4, j2)]; copied into ybw [j1, t, j2] fp16.
  4. ybw is stored as contiguous rows into y2[b, j1, t, j2]; the host
     transposes to [t, j1, j2] and upcasts to fp32.

Stage-2 matmuls are emitted with a 3-chunk lag behind stage 1 so the PE
never stalls on the PSUM->SBUF copy of the chunk it consumes.

Sharding: data-parallel over tokens, 256 tokens per core x 8 cores.
"""

import sys

if "/opt/trn_rl_repo" not in sys.path:
    sys.path.insert(0, "/opt/trn_rl_repo")

import numpy as np

import concourse.bacc as bacc
import concourse.mybir as mybir
from concourse import tile
from concourse.bass_utils import run_bass_kernel_spmd

F16 = mybir.dt.float16
F32 = mybir.dt.float32

N_CORES = 8
TOKENS = 2048
D = 16384  # 128 * 128
T_CORE = TOKENS // N_CORES  # 256
BLK = 128  # tokens per block (PE partition width)


def build_nc(n_tokens=T_CORE, lsub=32, quad=8, lag=2, sstore=32):
    """Build + compile the per-core program.

    lsub: tokens per xbar transpose-load call.
    quad: stage-1 matmuls per PSUM tile / tokens per stage-2 matmul.
    lag: stage-2 chunk lag behind stage 1 (PE pipelining).
    sstore: tokens per sub-store.
    """
    assert n_tokens % BLK == 0 and BLK % lsub == 0 and BLK % quad == 0
    assert quad % 4 == 0  # stage-2 works in 4-token (N=512) matmuls
    n_blk = n_tokens // BLK
    nq = BLK // quad
    nc = bacc.Bacc("TRN2", target_bir_lowering=False, debug=False)
    x = nc.dram_tensor("x", [n_tokens, D], F16, kind="ExternalInput")
    u = nc.dram_tensor("u", [128, 128], F16, kind="ExternalInput")
    v = nc.dram_tensor("v", [128, 128], F16, kind="ExternalInput")
    # y2[b, j1, t, j2]; host transposes to [t, j1, j2]
    y2 = nc.dram_tensor("y2", [n_blk, 128, BLK, 128], F16, kind="ExternalOutput")

    with tile.TileContext(nc) as tc:
        with (
            tc.tile_pool(name="const", bufs=1) as cpool,
            tc.tile_pool(name="xT", bufs=2) as xpool,
            tc.tile_pool(name="zb", bufs=2) as zpool,
            tc.tile_pool(name="yw", bufs=2) as ypool,
            tc.tile_pool(name="pa", bufs=2, space="PSUM") as papool,
            tc.tile_pool(name="pb", bufs=2, space="PSUM") as pbpool,
        ):
            u_sb = cpool.tile([128, 128], F16)
            v_sb = cpool.tile([128, 128], F16)
            nc.scalar.dma_start(u_sb[:], u[:])
            nc.scalar.dma_start(v_sb[:], v[:])

            # Hoist ALL xbar transpose-loads up front: tile serializes
            # DMA_TRANSPOSE against other in-flight DMAs (deadlock guard),
            # so loads emitted after stores would queue behind them.
            # xT[i2, i1, t] = x[t0+t, i1*128+i2]
            xTs = []
            for b in range(n_blk):
                t0 = b * BLK
                xT = xpool.tile([128, 128, BLK], F16)
                if b == 0:
                    # fine-grained first chunks so the PE starts early
                    subs = [16, 16, 32, 64]
                else:
                    subs = [lsub] * (BLK // lsub)
                soff = 0
                for sl in subs:
                    nc.sync.dma_start(
                        xT[:, :, soff : soff + sl],
                        x[t0 + soff : t0 + soff + sl, :],
                        transpose=True,
                    )
                    soff += sl
                xTs.append(xT)

            for b in range(n_blk):
                t0 = b * BLK
                xT = xTs[b]
                zb = zpool.tile([128, BLK, 128], F16)  # [i1, t, j2]
                yw = ypool.tile([128, BLK, 128], F16)  # [j1, t, j2]

                def stage2(c):
                    # one stage-2 "chunk" = quad tokens, as quad//4 matmuls of
                    # N=512 (PSUM bank limit) sharing one PSUM tile + one copy
                    nmm = quad // 4
                    pb = pbpool.tile([128, nmm, 512], F32)
                    for k in range(nmm):
                        tk = c * quad + k * 4
                        nc.tensor.matmul(
                            pb[:, k, :],
                            lhsT=u_sb[:],
                            rhs=zb[:, tk : tk + 4, :].rearrange(
                                "i1 t j2 -> i1 (t j2)"
                            ),
                            start=True,
                            stop=True,
                        )
                    # both PSUM-capable engines drain half the tile each
                    half = quad // 2
                    dst0 = yw[:, c * quad : c * quad + half, :].rearrange(
                        "j1 t j2 -> j1 (t j2)"
                    )
                    dst1 = yw[:, c * quad + half : (c + 1) * quad, :].rearrange(
                        "j1 t j2 -> j1 (t j2)"
                    )
                    flat = pb[:].rearrange("p a b -> p (a b)")
                    nc.scalar.copy(dst0, flat[:, : half * 128])
                    nc.vector.tensor_copy(dst1, flat[:, half * 128 :])
                    # sub-store once a full sstore-token span of yw is done
                    tend = (c + 1) * quad
                    if tend % sstore == 0:
                        s0 = tend - sstore
                        nc.gpsimd.dma_start(
                            y2[b, :, s0:tend, :], yw[:, s0:tend, :]
                        )

                for q in range(nq):
                    pa = papool.tile([128, quad, 128], F32)
                    for j in range(quad):
                        nc.tensor.matmul(
                            pa[:, j, :],
                            lhsT=xT[:, :, q * quad + j],
                            rhs=v_sb[:],
                            start=True,
                            stop=True,
                        )
                    half = quad // 2
                    nc.vector.tensor_copy(
                        zb[:, q * quad : q * quad + half, :], pa[:, :half, :]
                    )
                    nc.scalar.copy(
                        zb[:, q * quad + half : (q + 1) * quad, :],
                        pa[:, half:, :],
                    )
                    if q >= lag:
                        stage2(q - lag)
                for c in range(nq - lag, nq):
                    stage2(c)
    nc.compile()
    return nc


_NC_CACHE = {}


def _get_nc(**kw):
    key = tuple(sorted(kw.items()))
    if key not in _NC_CACHE:
        _NC_CACHE[key] = build_nc(**kw)
    return _NC_CACHE[key]


def run(x, U, V, lsub=32, quad=8, lag=2, sstore=32, trace=False, **spmd_kwargs):
    """Shard over 8 cores, run, gather. Returns (y_full, BassKernelResults)."""
    x = np.asarray(x).astype(np.float16)
    U = np.asarray(U).astype(np.float16)
    V = np.asarray(V).astype(np.float16)
    t_core = x.shape[0] // N_CORES
    nc = _get_nc(n_tokens=t_core, lsub=lsub, quad=quad, lag=lag, sstore=sstore)
    in_maps = [
        {"x": x[i * t_core : (i + 1) * t_core], "u": U, "v": V}
        for i in range(N_CORES)
    ]
    res = run_bass_kernel_spmd(
        nc, in_maps, list(range(N_CORES)), trace=trace, **spmd_kwargs
    )
    # y2[b, j1, t, j2] -> y[t, (j1 j2)]
    out = np.concatenate([res.results[i]["y2"] for i in range(N_CORES)], axis=0)
    out = (
        out.transpose(0, 2, 1, 3)
        .reshape(TOKENS, D)
        .astype(np.float32)
    )
    return out, res


def kernel(x, U, V):
    out, _ = run(x, U, V)
    return out


# revision 20
# speedup vs baseline: 1.1505x; 1.0457x over previous
"""Trainium2 Bass kernel for KroneckerLinear: y = x @ kron(U, V).

Math: with x[t] reshaped to X_t [i1=128, i2=128] (i2 contiguous) and
y[t] reshaped to Y_t [j1=128, j2=128] (j2 contiguous):

    Y_t = U^T @ X_t @ V

Dataflow (fp16 end to end, fp32 PSUM accumulation; the 2e-2 rel-err gate
leaves ~30x margin over fp16 rounding):

  1. x is cast to fp16 on the host.  Each 128-token block is loaded with
     the DMA xbar transpose straight from DRAM: x[t-block, :] (contiguous
     rows in HBM) lands in SBUF as xT[i2, i1, t] - no strided 512B
     descriptors anywhere.
  2. Stage 1 (V), per token:  lhsT = xT[:, :, t] = X_t^T [i2, i1],
     rhs = V [i2, j2]  ->  Z_t [i1, j2] in PSUM; copied/cast (DVE and ACT
     alternate) into zb [i1, t, j2] fp16, contiguous on both sides.
  3. Stage 2 (U), per 4 tokens:  lhsT = U [i1, j1] (shared stationary),
     rhs = zb[:, t4, :] [i1, 512] contiguous moving
     ->  PSUM [j1, (t4, j2)]; copied into ybw [j1, t, j2] fp16.
  4. ybw is stored as contiguous rows into y2[b, j1, t, j2]; the host
     transposes to [t, j1, j2] and upcasts to fp32.

Stage-2 matmuls are emitted with a 3-chunk lag behind stage 1 so the PE
never stalls on the PSUM->SBUF copy of the chunk it consumes.

Sharding: data-parallel over tokens, 256 tokens per core x 8 cores.
"""

import sys

if "/opt/trn_rl_repo" not in sys.path:
    sys.path.insert(0, "/opt/trn_rl_repo")

import numpy as np

import concourse.bacc as bacc
import concourse.mybir as mybir
from concourse import tile
from concourse.bass_utils import run_bass_kernel_spmd

F16 = mybir.dt.float16
F32 = mybir.dt.float32

N_CORES = 8
TOKENS = 2048
D = 16384  # 128 * 128
T_CORE = TOKENS // N_CORES  # 256
BLK = 128  # tokens per block (PE partition width)


def build_nc(n_tokens=T_CORE, lsub=32, quad=8, lag=2, sstore=32):
    """Build + compile the per-core program.

    lsub: tokens per xbar transpose-load call.
    quad: stage-1 matmuls per PSUM tile / tokens per stage-2 matmul.
    lag: stage-2 chunk lag behind stage 1 (PE pipelining).
    sstore: tokens per sub-store.
    """
    assert n_tokens % BLK == 0 and BLK % lsub == 0 and BLK % quad == 0
    assert quad % 4 == 0  # stage-2 works in 4-token (N=512) matmuls
    n_blk = n_tokens // BLK
    nq = BLK // quad
    nc = bacc.Bacc("TRN2", target_bir_lowering=False, debug=False)
    x = nc.dram_tensor("x", [n_tokens, D], F16, kind="ExternalInput")
    u = nc.dram_tensor("u", [128, 128], F16, kind="ExternalInput")
    v = nc.dram_tensor("v", [128, 128], F16, kind="ExternalInput")
    # y2[b, j1, t, j2]; host transposes to [t, j1, j2]
    y2 = nc.dram_tensor("y2", [n_blk, 128, BLK, 128], F16, kind="ExternalOutput")

    with tile.TileContext(nc) as tc:
        with (
            tc.tile_pool(name="const", bufs=1) as cpool,
            tc.tile_pool(name="xT", bufs=2) as xpool,
            tc.tile_pool(name="zb", bufs=2) as zpool,
            tc.tile_pool(name="yw", bufs=2) as ypool,
            tc.tile_pool(name="pa", bufs=2, space="PSUM") as papool,
            tc.tile_pool(name="pb", bufs=2, space="PSUM") as pbpool,
        ):
            u_sb = cpool.tile([128, 128], F16)
            v_sb = cpool.tile([128, 128], F16)
            nc.scalar.dma_start(u_sb[:], u[:])
            nc.scalar.dma_start(v_sb[:], v[:])

            # Hoist ALL xbar transpose-loads up front: tile serializes
            # DMA_TRANSPOSE against other in-flight DMAs (deadlock guard),
            # so loads emitted after stores would queue behind them.
            # xT[i2, i1, t] = x[t0+t, i1*128+i2]
            xTs = []
            for b in range(n_blk):
                t0 = b * BLK
                xT = xpool.tile([128, 128, BLK], F16)
                if b == 0:
                    # fine-grained first chunks so the PE starts early
                    subs = [16, 16, 32, 64]
                else:
                    subs = [lsub] * (BLK // lsub)
                soff = 0
                for sl in subs:
                    nc.sync.dma_start(
                        xT[:, :, soff : soff + sl],
                        x[t0 + soff : t0 + soff + sl, :],
                        transpose=True,
                    )
                    soff += sl
                xTs.append(xT)

            for b in range(n_blk):
                t0 = b * BLK
                xT = xTs[b]
                zb = zpool.tile([128, BLK, 128], F16)  # [i1, t, j2]
                yw = ypool.tile([128, BLK, 128], F16)  # [j1, t, j2]

                def stage2(c):
                    # one stage-2 "chunk" = quad tokens, as quad//4 matmuls of
                    # N=512 (PSUM bank limit) sharing one PSUM tile + one copy
                    nmm = quad // 4
                    pb = pbpool.tile([128, nmm, 512], F32)
                    for k in range(nmm):
                        tk = c * quad + k * 4
                        nc.tensor.matmul(
                            pb[:, k, :],
                            lhsT=u_sb[:],
                            rhs=zb[:, tk : tk + 4, :].rearrange(
                                "i1 t j2 -> i1 (t j2)"
                            ),
                            start=True,
                            stop=True,
                        )
                    dst = yw[:, c * quad : (c + 1) * quad, :].rearrange(
                        "j1 t j2 -> j1 (t j2)"
                    )
                    flat = pb[:].rearrange("p a b -> p (a b)")
                    if c % 2 == 0:
                        nc.scalar.copy(dst, flat)
                    else:
                        nc.vector.tensor_copy(dst, flat)
                    # sub-store once a full sstore-token span of yw is done
                    tend = (c + 1) * quad
                    if tend % sstore == 0:
                        s0 = tend - sstore
                        nc.gpsimd.dma_start(
                            y2[b, :, s0:tend, :], yw[:, s0:tend, :]
                        )

                for q in range(nq):
                    pa = papool.tile([128, quad, 128], F32)
                    for j in range(quad):
                        nc.tensor.matmul(
                            pa[:, j, :],
                            lhsT=xT[:, :, q * quad + j],
                            rhs=v_sb[:],
                            start=True,
                            stop=True,
                        )
                    dst = zb[:, q * quad : (q + 1) * quad, :]
                    if q % 2 == 1:
                        nc.scalar.copy(dst, pa[:])
                    else:
                        nc.vector.tensor_copy(dst, pa[:])
                    if q >= lag:
                        stage2(q - lag)
                for c in range(nq - lag, nq):
                    stage2(c)
    nc.compile()
    return nc


_NC_CACHE = {}


def _get_nc(**kw):
    key = tuple(sorted(kw.items()))
    if key not in _NC_CACHE:
        _NC_CACHE[key] = build_nc(**kw)
    return _NC_CACHE[key]


def run(x, U, V, lsub=32, quad=8, lag=2, sstore=32, trace=False, **spmd_kwargs):
    """Shard over 8 cores, run, gather. Returns (y_full, BassKernelResults)."""
    x = np.asarray(x).astype(np.float16)
    U = np.asarray(U).astype(np.float16)
    V = np.asarray(V).astype(np.float16)
    t_core = x.shape[0] // N_CORES
    nc = _get_nc(n_tokens=t_core, lsub=lsub, quad=quad, lag=lag, sstore=sstore)
    in_maps = [
        {"x": x[i * t_core : (i + 1) * t_core], "u": U, "v": V}
        for i in range(N_CORES)
    ]
    res = run_bass_kernel_spmd(
        nc, in_maps, list(range(N_CORES)), trace=trace, **spmd_kwargs
    )
    # y2[b, j1, t, j2] -> y[t, (j1 j2)]
    out = np.concatenate([res.results[i]["y2"] for i in range(N_CORES)], axis=0)
    out = (
        out.transpose(0, 2, 1, 3)
        .reshape(TOKENS, D)
        .astype(np.float32)
    )
    return out, res


def kernel(x, U, V):
    out, _ = run(x, U, V)
    return out
